# revision 66
# baseline (speedup 1.0000x reference)
"""Trainium2 Bass kernel: block-causal cross attention (CrossCausalAttention).

Full-input contract: kernel(**inputs) takes the unsharded tensors from
setup_inputs() and returns the full [v, b, c, h, w] output.

Sharding: 8 NeuronCores = 4 batches (data parallel) x 2 head-groups of 4
heads (tensor parallel).  Each core computes a partial y^T [512, 2048] for
its (batch, head-group); the host sums the two head-group partials per batch.

v25 schedule (ScalarE-exp is the pacer; everything else hides under it):
  - Q/K projection fillers are split into TWO half-thunks of 2 matmuls
    each (same key, adjacent in queue) so a single filler pop never
    exceeds the per-k-tile PE budget.
  - Filler thunks are SKIPPED on iterations whose emit_AV lazily projects
    a new V tile: per k-tile the PE budget vs exp is ~1.15us, and
    V-projection (+1us) plus a filler together overload the iteration,
    blow the 2-slot S ring and stall the exp stream.
  - Host pre-arranges x/out to [128p, 4ch, 4ci|4co, 512t] so every chunk
    DMA is one contiguous 4KB-per-partition transfer (128 descriptors;
    DMA-issue time was the startup critical path).
  - No warmup matmuls: stage-0 projections start the moment chunk-0 DMA
    lands; V tiles project lazily inside the attention loop.
  - Attention runs S-matmuls 2 k-tiles ahead of the exp stream, AV one
    behind; e-tile pool is 6 deep so AV jitter never stalls exp.
  - Projections for chunk c+1 and out-proj for chunk c-1 are filler
    thunks popped one per 2 k-tiles inside the attention loop.
  - Each block's PSUM evacuation (O-rows, D-rows, reciprocal, bcast,
    normalize) is DEFERRED into the next block's instruction stream so
    its serial DVE/gpsimd chain overlaps attention instead of stalling
    the chunk boundary; the final block's evac splits the copies onto
    the then-idle ScalarE.
  - Tail: the last chunk's out-proj is p-split - all four p=0 half
    matmuls (needing only A3's already-normalized oT) queue ahead of the
    B3 evac chain and overlap it in free ps_s PSUM; the out DMA leaves
    in two queue-parallel halves.
"""
import sys
from collections import deque

for _p in ("/opt/trn_rl_repo", "/root/.axon_site/_ro/trn_rl_repo"):
    if _p not in sys.path:
        sys.path.append(_p)

import ml_dtypes
import numpy as np

import concourse.bass as bass  # noqa: E402,F401
import concourse.mybir as mybir  # noqa: E402
import concourse.tile as tile  # noqa: E402
from concourse import bacc  # noqa: E402
from concourse.bass_utils import run_bass_kernel_spmd  # noqa: E402

F32 = mybir.dt.float32
BF16 = mybir.dt.bfloat16
BF16_NP = ml_dtypes.bfloat16

V, C, HW = 8, 512, 256
T = V * HW                 # 2048
NHC = 4                    # heads per core
HD = 64
GC = NHC * HD              # 256 channels per head-group
NKT = T // 128             # 16 tk tiles
VW = NHC * (HD + 1)        # 260


def _build(nc):
    from contextlib import ExitStack

    # x layouts are host-prearranged to [128p, 4ch, 4ci, 512t] so each
    # t-chunk DMA is a single contiguous 4KB-per-partition transfer
    # (128 descriptors instead of 512 - DMA issue was the startup
    # critical path).  Same for the output: [128p, 4ch, 4co, 512t].
    xq = nc.dram_tensor("xq", [128, 4 * T], BF16, kind="ExternalInput")
    xkv = nc.dram_tensor("xkv", [128, 4 * T], BF16, kind="ExternalInput")
    wq = nc.dram_tensor("wq", [128, 4 * GC], BF16, kind="ExternalInput")
    wk = nc.dram_tensor("wk", [128, 4 * GC], BF16, kind="ExternalInput")
    wv = nc.dram_tensor("wv", [128, 4 * GC], BF16, kind="ExternalInput")
    wp = nc.dram_tensor("wp", [128, 2 * C], BF16, kind="ExternalInput")
    out = nc.dram_tensor("out", [128, 4 * T], BF16, kind="ExternalOutput")

    with tile.TileContext(nc) as tc, ExitStack() as ctx:
        persist = ctx.enter_context(tc.tile_pool(name="persist", bufs=1))
        epool = ctx.enter_context(tc.tile_pool(name="e", bufs=6))
        rpool = ctx.enter_context(tc.tile_pool(name="r", bufs=4))
        evac = ctx.enter_context(tc.tile_pool(name="evac", bufs=2))
        ps_s = ctx.enter_context(tc.tile_pool(name="ps_s", bufs=2, space="PSUM"))
        po_p = ctx.enter_context(tc.tile_pool(name="po", bufs=2, space="PSUM"))
        fill = ctx.enter_context(tc.tile_pool(name="fill", bufs=2, space="PSUM"))

        # ---- weights: host already laid out [128, 4, GC] / [128, 2, C] ----
        wk_sb = persist.tile([128, 4 * GC], BF16, tag="wk", name="wk_sb")
        nc.scalar.dma_start(wk_sb[:], wk[:])
        wq_sb = persist.tile([128, 4 * GC], BF16, tag="wq", name="wq_sb")
        nc.scalar.dma_start(wq_sb[:], wq[:])
        wv_sb = persist.tile([128, 4 * GC], BF16, tag="wv", name="wv_sb")
        nc.scalar.dma_start(wv_sb[:], wv[:])
        wp_sb = persist.tile([128, 2 * C], BF16, tag="wp", name="wp_sb")
        nc.scalar.dma_start(wp_sb[:], wp[:])

        def wslice(t, ci, lo, hi, cols=GC):
            return t[:, ci * cols + lo: ci * cols + hi]

        # ---- inputs: [128, (ci, T)]; 4 t-chunk DMAs per tensor ----
        xkvT = persist.tile([128, 4 * T], BF16, tag="xkvT", name="xkvT")
        xqT = persist.tile([128, 4 * T], BF16, tag="xqT", name="xqT")
        for c in range(4):
            lo, hi = c * 2048, (c + 1) * 2048
            nc.sync.dma_start(xkvT[:, lo:hi], xkv[:, lo:hi])
            nc.sync.dma_start(xqT[:, lo:hi], xq[:, lo:hi])

        def xslice(x_sb, ci, ch):
            base = ch * 2048 + ci * 512
            return x_sb[:, base: base + 512]

        qT = [persist.tile([128, T], BF16, tag=f"qT{p}", name=f"qT{p}")
              for p in range(2)]
        kT = [persist.tile([128, T], BF16, tag=f"kT{p}", name=f"kT{p}")
              for p in range(2)]
        v_sb = [persist.tile([128, VW], BF16, tag=f"v{tk}", name=f"v{tk}")
                for tk in range(NKT)]
        oT = [persist.tile([128, T], BF16, tag=f"oT{p}", name=f"oT{p}")
              for p in range(2)]

        # ---- per-chunk projection / V-tile emitters ----
        def proj_qk(w_sb, x_sb, dst, p, ch):
            ps = fill.tile([128, 512], F32, tag="fill", name=f"ps_p{p}c{ch}")
            for ci in range(4):
                nc.tensor.matmul(
                    ps[:],
                    wslice(w_sb, ci, p * 128, (p + 1) * 128),
                    xslice(x_sb, ci, ch),
                    start=(ci == 0), stop=(ci == 3),
                )
            nc.vector.tensor_copy(dst[:, ch * 512:(ch + 1) * 512], ps[:])

        def proj_v(tk):
            vt = v_sb[tk]
            nc.vector.memset(
                vt[:].rearrange("p (h x) -> p h x", x=65)[:, :, 64:65]
                .bitcast(mybir.dt.uint16),
                0x3F80)  # bf16 bits of 1.0
            ps = fill.tile([128, 512], F32, tag="fill", name=f"ps_v{tk}")
            for ci in range(4):
                base = (tk // 4) * 2048 + ci * 512 + (tk % 4) * 128
                nc.tensor.matmul(
                    ps[:, 0:GC],
                    xkvT[:, base: base + 128],
                    wslice(wv_sb, ci, 0, GC),
                    start=(ci == 0), stop=(ci == 3),
                )
            nc.vector.tensor_copy(
                vt[:].rearrange("p (h x) -> p h x", x=65)[:, :, 0:64],
                ps[:, 0:GC].rearrange("p (h x) -> p h x", x=64),
            )

        # ---- attention: S runs 2 k-tiles ahead of exp, AV one behind ----
        v_done = [False] * NKT

        def ensure_v(tk):
            if not v_done[tk]:
                proj_v(tk)
                v_done[tk] = True

        def attention_block(p, qc, pq, oq, pre_evac=None, last=False):
            hA, hB = 2 * p, 2 * p + 1
            q0 = qc * 512
            nfull = 2 * (2 * qc + 1)
            nk = nfull + 2
            po = [po_p.tile([65, 512], F32, tag="po", name=f"po{p}q{qc}{i}")
                  for i in range(2)]
            sq = [None] * nk
            es = [None] * nk

            def emit_S(kb):
                sq[kb] = ps_s.tile([128, 1024], F32, tag="s", name="sps")
                for i, h0 in enumerate((0, 64)):
                    nc.tensor.matmul(
                        sq[kb][:, i * 512:(i + 1) * 512],
                        kT[p][h0:h0 + 64, kb * 128:(kb + 1) * 128],
                        qT[p][h0:h0 + 64, q0:q0 + 512],
                        start=True, stop=True,
                    )

            def emit_ACT(kb):
                if kb < nfull:
                    e = epool.tile([128, 1024], BF16, tag="e", name="e")
                    nc.scalar.activation(
                        e[:], sq[kb][:],
                        mybir.ActivationFunctionType.Exp, scale=0.125)
                else:
                    # boundary key block: only the 2nd half of the queries
                    # may attend.  (K=64 row-base-64 matmuls with N=256
                    # hang TRN2, so S is computed at N=512 and the valid
                    # halves are gathered by the exp's access pattern.)
                    e = epool.tile([128, 512], BF16, tag="e", name="eb")
                    nc.scalar.activation(
                        e[:].rearrange("p (h x) -> p h x", x=256),
                        sq[kb][:].rearrange("p (h x) -> p h x",
                                            x=512)[:, :, 256:512],
                        mybir.ActivationFunctionType.Exp, scale=0.125)
                es[kb] = e
                sq[kb] = None

            def emit_AV(kb):
                ensure_v(kb)
                e = es[kb]
                if kb < nfull:
                    rhs = [e[:, 0:512], e[:, 512:1024]]
                    dst = [po[0][:], po[1][:]]
                else:
                    rhs = [e[:, 0:256], e[:, 256:512]]
                    dst = [po[0][:, 256:512], po[1][:, 256:512]]
                for i, h in enumerate((hA, hB)):
                    nc.tensor.matmul(
                        dst[i],
                        v_sb[kb][:, h * 65: h * 65 + 65],
                        rhs[i],
                        start=(kb == 0), stop=(kb == nk - 1),
                        skip_group_check=True,
                    )
                es[kb] = None

            # selective drain: emit exactly the queued projections this
            # block's S stream depends on; leave the rest for pacing
            need = {("k", p, cc) for cc in range(qc + 1)}
            need.add(("q", p, qc))
            rest = deque()
            while pq:
                key, th = pq.popleft()
                if key in need:
                    th()
                else:
                    rest.append((key, th))
            pq.extend(rest)

            emit_S(0)
            emit_S(1)
            emit_ACT(0)
            # the PREVIOUS block's PSUM evacuation goes here: its serial
            # DVE/gpsimd chain overlaps this block's S/exp stream instead
            # of stalling the chunk boundary (po WAR is released before
            # this block's AV(0) needs the buffers)
            if pre_evac is not None:
                pre_evac()
            for kb in range(nk):
                if kb + 2 < nk:
                    emit_S(kb + 2)
                if kb + 1 < nk:
                    emit_ACT(kb + 1)
                # iterations whose emit_AV lazily projects a new V tile
                # already carry ~1us of extra PE work - adding a filler
                # too overloads the iteration and stalls the exp stream.
                # Projection HALF-thunks (~0.45us) fit the remaining
                # budget every iteration; heavier out-proj thunks only
                # every 2nd k-tile.
                if v_done[kb] and pq:
                    pq.popleft()[1]()
                elif v_done[kb] and kb % 2 == 1 and oq:
                    oq.popleft()()
                emit_AV(kb)

            # deferred PSUM evacuation (run by the NEXT block): O rows cast
            # to bf16 at their target partition offset, both heads' D rows
            # packed into partition 0 (custom ops need base-0 inputs) and
            # inverted in ONE reciprocal; bcast on gpsimd; normalizing
            # multiplies on DVE with partition-aligned operands
            def evac_thunk(p=p, q0=q0, po=po, last=last):
                drow2 = rpool.tile([1, 1024], F32, tag="drow", name="drow2")
                ous = []
                for i in range(2):
                    ou = rpool.tile([128, 512], BF16, tag=f"ou{i}", name="ou")
                    if last:
                        # ScalarE is idle after the final exp: it handles
                        # the O-row evacuation while DVE does D-rows+recip
                        nc.scalar.copy(
                            ou[i * 64:(i + 1) * 64, :], po[i][0:64, :])
                        nc.scalar.copy(
                            drow2[0:1, i * 512:(i + 1) * 512],
                            po[i][64:65, :])
                    else:
                        nc.vector.tensor_copy(
                            ou[i * 64:(i + 1) * 64, :], po[i][0:64, :])
                        nc.vector.tensor_copy(
                            drow2[0:1, i * 512:(i + 1) * 512],
                            po[i][64:65, :])
                    ous.append(ou)
                rcp2 = rpool.tile([1, 1024], F32, tag="rcp", name="rcp2")
                nc.vector.reciprocal_approx_fast(rcp2[:], drow2[:])
                rb = rpool.tile([128, 1024], F32, tag="rb", name="rb")
                nc.gpsimd.partition_broadcast(rb[:], rcp2[0:1, :])
                for i in range(2):
                    nc.vector.tensor_mul(
                        oT[p][i * 64:(i + 1) * 64, q0:q0 + 512],
                        ous[i][i * 64:(i + 1) * 64, :],
                        rb[i * 64:(i + 1) * 64, i * 512:(i + 1) * 512])
            return evac_thunk

        def outproj_co(ch, co, ych):
            ps = fill.tile([128, 512], F32, tag="fill", name="ps_out")
            for p in range(2):
                nc.tensor.matmul(
                    ps[:],
                    wp_sb[:, p * C + co * 128: p * C + (co + 1) * 128],
                    oT[p][:, ch * 512:(ch + 1) * 512],
                    start=(p == 0), stop=(p == 1),
                )
            nc.vector.tensor_copy(ych[:, co * 512:(co + 1) * 512], ps[:])

        def outproj_thunks(ch):
            ych = evac.tile([128, T], BF16, tag="y", name=f"ych{ch}")
            thunks = []
            for co in range(4):
                def mk(co=co):
                    def t():
                        outproj_co(ch, co, ych)
                        if ch == 3 and co == 1:
                            # final chunk: first half leaves early on the
                            # idle sync queue so the last transfer is short
                            nc.sync.dma_start(
                                out[:, ch * 2048: ch * 2048 + 1024],
                                ych[:, 0:1024])
                        elif ch == 3 and co == 3:
                            nc.gpsimd.dma_start(
                                out[:, ch * 2048 + 1024: ch * 2048 + 2048],
                                ych[:, 1024:2048])
                        elif co == 3:
                            nc.gpsimd.dma_start(
                                out[:, ch * 2048:(ch + 1) * 2048], ych[:])
                    return t
                thunks.append(mk())
            return thunks

        # ---- stage 0 pre-work (paced by chunk-0 DMA arrival); only what
        # attnA0 needs — K1/Q1 ride the filler queue ----
        with nc.named_scope("stage0"):
            proj_qk(wk_sb, xkvT, kT[0], 0, 0)
            proj_qk(wq_sb, xqT, qT[0], 0, 0)

        pq = deque()
        oq = deque()

        def proj_halves(key, w_sb, x_sb, dst, p, ch):
            """A projection as TWO filler thunks of 2 matmuls each (same
            key, adjacent in the queue - need-drain runs both in order)
            so one filler pop never exceeds the per-k-tile PE budget."""
            st = {}

            def t1():
                st["ps"] = fill.tile([128, 512], F32, tag="fill",
                                     name=f"ph{p}{ch}")
                for ci in range(2):
                    nc.tensor.matmul(
                        st["ps"][:],
                        wslice(w_sb, ci, p * 128, (p + 1) * 128),
                        xslice(x_sb, ci, ch),
                        start=(ci == 0), stop=False, skip_group_check=True,
                    )

            def t2():
                for ci in range(2, 4):
                    nc.tensor.matmul(
                        st["ps"][:],
                        wslice(w_sb, ci, p * 128, (p + 1) * 128),
                        xslice(x_sb, ci, ch),
                        start=False, stop=(ci == 3), skip_group_check=True,
                    )
                nc.vector.tensor_copy(
                    dst[:, ch * 512:(ch + 1) * 512], st["ps"][:])

            return [(key, t1), (key, t2)]

        pq.extend(proj_halves(("k", 1, 0), wk_sb, xkvT, kT[1], 1, 0))
        pq.extend(proj_halves(("q", 1, 0), wq_sb, xqT, qT[1], 1, 0))

        def stage_thunks(cn):
            return (
                proj_halves(("k", 0, cn), wk_sb, xkvT, kT[0], 0, cn)
                + proj_halves(("k", 1, cn), wk_sb, xkvT, kT[1], 1, cn)
                + proj_halves(("q", 0, cn), wq_sb, xqT, qT[0], 0, cn)
                + proj_halves(("q", 1, cn), wq_sb, xqT, qT[1], 1, cn)
            )

        ev = None
        for c in range(4):
            if c < 3:
                pq.extend(stage_thunks(c + 1))
            with nc.named_scope(f"attnA{c}"):
                ev = attention_block(0, c, pq, oq, pre_evac=ev)
            with nc.named_scope(f"attnB{c}"):
                ev = attention_block(1, c, pq, oq, pre_evac=ev,
                                     last=(c == 3))
            if c < 3:
                oq.extend(outproj_thunks(c))
        # tail: the final chunk's out-proj is split by p so all four p=0
        # halves (only need A3's oT, normalized during B3) sit ready in
        # the PE queue and run concurrently with the B3 evac chain; the
        # p=1 halves + casts follow once the chain's muls land.  PSUM
        # comes from the now-free ps_s pool.
        with nc.named_scope("tail"):
            while oq:
                oq.popleft()()
            ych = evac.tile([128, T], BF16, tag="y", name="ych3f")
            pst = [ps_s.tile([128, 1024], F32, tag="s", name=f"pso{j}")
                   for j in range(2)]

            def ps_slice(co):
                return pst[co // 2][:, (co % 2) * 512:(co % 2 + 1) * 512]

            for co in range(4):
                nc.tensor.matmul(
                    ps_slice(co),
                    wp_sb[:, 0 * C + co * 128: 0 * C + (co + 1) * 128],
                    oT[0][:, 3 * 512:4 * 512],
                    start=True, stop=False, skip_group_check=True,
                )
            ev()
            for co in range(4):
                nc.tensor.matmul(
                    ps_slice(co),
                    wp_sb[:, 1 * C + co * 128: 1 * C + (co + 1) * 128],
                    oT[1][:, 3 * 512:4 * 512],
                    start=False, stop=True, skip_group_check=True,
                )
                nc.vector.tensor_copy(
                    ych[:, co * 512:(co + 1) * 512], ps_slice(co))
                if co == 1:
                    nc.sync.dma_start(
                        out[:, 3 * 2048: 3 * 2048 + 1024], ych[:, 0:1024])
            nc.gpsimd.dma_start(
                out[:, 3 * 2048 + 1024: 4 * 2048], ych[:, 1024:2048])
    return nc


_NC_CACHE = None


def _get_nc():
    global _NC_CACHE
    if _NC_CACHE is None:
        nc = bacc.Bacc("TRN2", target_bir_lowering=False, debug=False,
                       num_devices=8)
        _build(nc)
        nc.compile()
        _NC_CACHE = nc
    return _NC_CACHE


def _shard_inputs(q, kv, Wq, Wkv, Wp):
    v, b, c, h, w = q.shape
    in_maps = []
    for bi in range(b):
        # [v, c, hw] -> [c, T] -> [128p, 4ch, 4ci, 512t] pre-arranged on
        # host so each on-device chunk DMA is contiguous per partition
        def xlay(x):
            xT = x.reshape(v, c, h * w).transpose(1, 0, 2).reshape(c, -1)
            return np.ascontiguousarray(
                xT.reshape(4, 128, 4, 512).transpose(1, 2, 0, 3)
                .reshape(128, -1)).astype(BF16_NP)
        xq = xlay(q[:, bi])
        xkv = xlay(kv[:, bi])
        for g in range(2):
            wq_h = np.ascontiguousarray(
                Wq[:, g * GC:(g + 1) * GC].reshape(4, 128, GC)
                .transpose(1, 0, 2).reshape(128, -1)).astype(BF16_NP)
            wk_h = np.ascontiguousarray(
                Wkv[:, g * GC:(g + 1) * GC].reshape(4, 128, GC)
                .transpose(1, 0, 2).reshape(128, -1)).astype(BF16_NP)
            wv_h = np.ascontiguousarray(
                Wkv[:, c + g * GC:c + (g + 1) * GC].reshape(4, 128, GC)
                .transpose(1, 0, 2).reshape(128, -1)).astype(BF16_NP)
            wp_h = np.ascontiguousarray(
                Wp[g * GC:(g + 1) * GC, :].reshape(2, 128, c)
                .transpose(1, 0, 2).reshape(128, -1)).astype(BF16_NP)
            in_maps.append({
                "xq": xq, "xkv": xkv,
                "wq": wq_h, "wk": wk_h, "wv": wv_h, "wp": wp_h,
            })
    return in_maps


def kernel(q, kv, Wq, bq, Wkv, bkv, Wp, bp, _trace=False):
    q = np.asarray(q, np.float32)
    kv = np.asarray(kv, np.float32)
    v, b, c, h, w = q.shape
    nc = _get_nc()
    in_maps = _shard_inputs(q, kv, np.asarray(Wq, np.float32),
                            np.asarray(Wkv, np.float32),
                            np.asarray(Wp, np.float32))
    res = run_bass_kernel_spmd(nc, in_maps, core_ids=list(range(8)),
                               trace=_trace)
    y = np.empty((v, b, c, h, w), np.float32)
    bp32 = np.asarray(bp, np.float32)
    for bi in range(b):
        y2 = (res.results[bi * 2]["out"].astype(np.float32)
              + res.results[bi * 2 + 1]["out"].astype(np.float32))
        # [128p, 4ch, 4co, 512t] -> [c = co*128+p, T = ch*512+t]
        yT = (y2.reshape(128, 4, 4, 512).transpose(2, 0, 1, 3)
              .reshape(c, v * h * w))
        yT = yT + bp32[:, None]
        y[:, bi] = yT.reshape(c, v, h, w).transpose(1, 0, 2, 3)
    kernel._last_exec_time_ns = res.exec_time_ns
    kernel._last_results = res
    return y


# revision 67
# speedup vs baseline: 1.0358x; 1.0358x over previous
"""Trainium2 Bass kernel: block-causal cross attention (CrossCausalAttention).

Full-input contract: kernel(**inputs) takes the unsharded tensors from
setup_inputs() and returns the full [v, b, c, h, w] output.

Sharding: 8 NeuronCores = 4 batches (data parallel) x 2 head-groups of 4
heads (tensor parallel).  Each core computes a partial y^T [512, 2048] for
its (batch, head-group); the host sums the two head-group partials per batch.

v25 schedule (ScalarE-exp is the pacer; everything else hides under it):
  - Q/K projection fillers are split into TWO half-thunks of 2 matmuls
    each (same key, adjacent in queue) so a single filler pop never
    exceeds the per-k-tile PE budget.
  - Filler thunks are SKIPPED on iterations whose emit_AV lazily projects
    a new V tile: per k-tile the PE budget vs exp is ~1.15us, and
    V-projection (+1us) plus a filler together overload the iteration,
    blow the 2-slot S ring and stall the exp stream.
  - Host pre-arranges x/out to [128p, 4ch, 4ci|4co, 512t] so every chunk
    DMA is one contiguous 4KB-per-partition transfer (128 descriptors;
    DMA-issue time was the startup critical path).
  - No warmup matmuls: stage-0 projections start the moment chunk-0 DMA
    lands; V tiles project lazily inside the attention loop.
  - Attention runs S-matmuls 2 k-tiles ahead of the exp stream, AV one
    behind; e-tile pool is 6 deep so AV jitter never stalls exp.
  - Projections for chunk c+1 and out-proj for chunk c-1 are filler
    thunks popped one per 2 k-tiles inside the attention loop.
  - Each block's PSUM evacuation (O-rows, D-rows, reciprocal, bcast,
    normalize) is DEFERRED into the next block's instruction stream so
    its serial DVE/gpsimd chain overlaps attention instead of stalling
    the chunk boundary; the final block's evac splits the copies onto
    the then-idle ScalarE.
  - Tail: the last chunk's out-proj is p-split - all four p=0 half
    matmuls (needing only A3's already-normalized oT) queue ahead of the
    B3 evac chain and overlap it in free ps_s PSUM; the out DMA leaves
    in two queue-parallel halves.
"""
import sys
from collections import deque

for _p in ("/opt/trn_rl_repo", "/root/.axon_site/_ro/trn_rl_repo"):
    if _p not in sys.path:
        sys.path.append(_p)

import ml_dtypes
import numpy as np

import concourse.bass as bass  # noqa: E402,F401
import concourse.mybir as mybir  # noqa: E402
import concourse.tile as tile  # noqa: E402
from concourse import bacc  # noqa: E402
from concourse.bass_utils import run_bass_kernel_spmd  # noqa: E402

F32 = mybir.dt.float32
BF16 = mybir.dt.bfloat16
BF16_NP = ml_dtypes.bfloat16

V, C, HW = 8, 512, 256
T = V * HW                 # 2048
NHC = 4                    # heads per core
HD = 64
GC = NHC * HD              # 256 channels per head-group
NKT = T // 128             # 16 tk tiles
VW = NHC * (HD + 1)        # 260


def _build(nc):
    from contextlib import ExitStack

    # x layouts are host-prearranged to [128p, 4ch, 4ci, 512t] so each
    # t-chunk DMA is a single contiguous 4KB-per-partition transfer
    # (128 descriptors instead of 512 - DMA issue was the startup
    # critical path).  Same for the output: [128p, 4ch, 4co, 512t].
    xq = nc.dram_tensor("xq", [128, 4 * T], BF16, kind="ExternalInput")
    xkv = nc.dram_tensor("xkv", [128, 4 * T], BF16, kind="ExternalInput")
    wq = nc.dram_tensor("wq", [128, 4 * GC], BF16, kind="ExternalInput")
    wk = nc.dram_tensor("wk", [128, 4 * GC], BF16, kind="ExternalInput")
    wv = nc.dram_tensor("wv", [128, 4 * GC], BF16, kind="ExternalInput")
    wp = nc.dram_tensor("wp", [128, 2 * C], BF16, kind="ExternalInput")
    out = nc.dram_tensor("out", [128, 4 * T], BF16, kind="ExternalOutput")

    with tile.TileContext(nc) as tc, ExitStack() as ctx:
        persist = ctx.enter_context(tc.tile_pool(name="persist", bufs=1))
        epool = ctx.enter_context(tc.tile_pool(name="e", bufs=6))
        rpool = ctx.enter_context(tc.tile_pool(name="r", bufs=4))
        evac = ctx.enter_context(tc.tile_pool(name="evac", bufs=2))
        ps_s = ctx.enter_context(tc.tile_pool(name="ps_s", bufs=2, space="PSUM"))
        po_p = ctx.enter_context(tc.tile_pool(name="po", bufs=2, space="PSUM"))
        fill = ctx.enter_context(tc.tile_pool(name="fill", bufs=2, space="PSUM"))

        # ---- weights: host already laid out [128, 4, GC] / [128, 2, C] ----
        wk_sb = persist.tile([128, 4 * GC], BF16, tag="wk", name="wk_sb")
        nc.scalar.dma_start(wk_sb[:], wk[:])
        wq_sb = persist.tile([128, 4 * GC], BF16, tag="wq", name="wq_sb")
        nc.scalar.dma_start(wq_sb[:], wq[:])
        wv_sb = persist.tile([128, 4 * GC], BF16, tag="wv", name="wv_sb")
        nc.scalar.dma_start(wv_sb[:], wv[:])
        wp_sb = persist.tile([128, 2 * C], BF16, tag="wp", name="wp_sb")
        nc.scalar.dma_start(wp_sb[:], wp[:])

        def wslice(t, ci, lo, hi, cols=GC):
            return t[:, ci * cols + lo: ci * cols + hi]

        # ---- inputs: [128, (ci, T)]; 4 t-chunk DMAs per tensor ----
        xkvT = persist.tile([128, 4 * T], BF16, tag="xkvT", name="xkvT")
        xqT = persist.tile([128, 4 * T], BF16, tag="xqT", name="xqT")
        for c in range(4):
            lo, hi = c * 2048, (c + 1) * 2048
            nc.sync.dma_start(xkvT[:, lo:hi], xkv[:, lo:hi])
            nc.sync.dma_start(xqT[:, lo:hi], xq[:, lo:hi])

        def xslice(x_sb, ci, ch):
            base = ch * 2048 + ci * 512
            return x_sb[:, base: base + 512]

        qT = [persist.tile([128, T], BF16, tag=f"qT{p}", name=f"qT{p}")
              for p in range(2)]
        kT = [persist.tile([128, T], BF16, tag=f"kT{p}", name=f"kT{p}")
              for p in range(2)]
        v_sb = [persist.tile([128, VW], BF16, tag=f"v{tk}", name=f"v{tk}")
                for tk in range(NKT)]
        oT = [persist.tile([128, T], BF16, tag=f"oT{p}", name=f"oT{p}")
              for p in range(2)]

        # ---- per-chunk projection / V-tile emitters ----
        def proj_qk(w_sb, x_sb, dst, p, ch):
            ps = fill.tile([128, 512], F32, tag="fill", name=f"ps_p{p}c{ch}")
            for ci in range(4):
                nc.tensor.matmul(
                    ps[:],
                    wslice(w_sb, ci, p * 128, (p + 1) * 128),
                    xslice(x_sb, ci, ch),
                    start=(ci == 0), stop=(ci == 3),
                )
            nc.vector.tensor_copy(dst[:, ch * 512:(ch + 1) * 512], ps[:])

        def proj_v(tk):
            vt = v_sb[tk]
            nc.vector.memset(
                vt[:].rearrange("p (h x) -> p h x", x=65)[:, :, 64:65]
                .bitcast(mybir.dt.uint16),
                0x3F80)  # bf16 bits of 1.0
            ps = fill.tile([128, 512], F32, tag="fill", name=f"ps_v{tk}")
            for ci in range(4):
                base = (tk // 4) * 2048 + ci * 512 + (tk % 4) * 128
                nc.tensor.matmul(
                    ps[:, 0:GC],
                    xkvT[:, base: base + 128],
                    wslice(wv_sb, ci, 0, GC),
                    start=(ci == 0), stop=(ci == 3),
                )
            nc.vector.tensor_copy(
                vt[:].rearrange("p (h x) -> p h x", x=65)[:, :, 0:64],
                ps[:, 0:GC].rearrange("p (h x) -> p h x", x=64),
            )

        # ---- attention: S runs 2 k-tiles ahead of exp, AV one behind ----
        v_done = [False] * NKT

        def ensure_v(tk):
            if not v_done[tk]:
                proj_v(tk)
                v_done[tk] = True

        def attention_block(p, qc, pq, oq, pre_evac=None, last=False):
            hA, hB = 2 * p, 2 * p + 1
            q0 = qc * 512
            nfull = 2 * (2 * qc + 1)
            nk = nfull + 2
            po = [po_p.tile([65, 512], F32, tag="po", name=f"po{p}q{qc}{i}")
                  for i in range(2)]
            sq = [None] * nk
            es = [None] * nk

            def emit_S(kb):
                sq[kb] = ps_s.tile([128, 1024], F32, tag="s", name="sps")
                for i, h0 in enumerate((0, 64)):
                    nc.tensor.matmul(
                        sq[kb][:, i * 512:(i + 1) * 512],
                        kT[p][h0:h0 + 64, kb * 128:(kb + 1) * 128],
                        qT[p][h0:h0 + 64, q0:q0 + 512],
                        start=True, stop=True,
                    )

            def emit_ACT(kb):
                if kb < nfull:
                    e = epool.tile([128, 1024], BF16, tag="e", name="e")
                    nc.scalar.activation(
                        e[:], sq[kb][:],
                        mybir.ActivationFunctionType.Exp, scale=0.125)
                else:
                    # boundary key block: only the 2nd half of the queries
                    # may attend.  (K=64 row-base-64 matmuls with N=256
                    # hang TRN2, so S is computed at N=512 and the valid
                    # halves are gathered by the exp's access pattern.)
                    e = epool.tile([128, 512], BF16, tag="e", name="eb")
                    nc.scalar.activation(
                        e[:].rearrange("p (h x) -> p h x", x=256),
                        sq[kb][:].rearrange("p (h x) -> p h x",
                                            x=512)[:, :, 256:512],
                        mybir.ActivationFunctionType.Exp, scale=0.125)
                es[kb] = e
                sq[kb] = None

            def emit_AV(kb):
                ensure_v(kb)
                e = es[kb]
                if kb < nfull:
                    rhs = [e[:, 0:512], e[:, 512:1024]]
                    dst = [po[0][:], po[1][:]]
                else:
                    rhs = [e[:, 0:256], e[:, 256:512]]
                    dst = [po[0][:, 256:512], po[1][:, 256:512]]
                for i, h in enumerate((hA, hB)):
                    nc.tensor.matmul(
                        dst[i],
                        v_sb[kb][:, h * 65: h * 65 + 65],
                        rhs[i],
                        start=(kb == 0), stop=(kb == nk - 1),
                        skip_group_check=True,
                    )
                es[kb] = None

            # selective drain: emit exactly the queued projections this
            # block's S stream depends on; leave the rest for pacing
            need = {("k", p, cc) for cc in range(qc + 1)}
            need.add(("q", p, qc))
            rest = deque()
            while pq:
                key, th = pq.popleft()
                if key in need:
                    th()
                else:
                    rest.append((key, th))
            pq.extend(rest)

            emit_S(0)
            emit_S(1)
            emit_ACT(0)
            # the PREVIOUS block's PSUM evacuation goes here: its serial
            # DVE/gpsimd chain overlaps this block's S/exp stream instead
            # of stalling the chunk boundary (po WAR is released before
            # this block's AV(0) needs the buffers)
            if pre_evac is not None:
                pre_evac()
            for kb in range(nk):
                if kb + 2 < nk:
                    emit_S(kb + 2)
                if kb + 1 < nk:
                    emit_ACT(kb + 1)
                # iterations whose emit_AV lazily projects a new V tile
                # already carry ~1us of extra PE work - adding a filler
                # too overloads the iteration and stalls the exp stream
                if v_done[kb] and kb % 2 == 1 and (pq or oq):
                    (pq.popleft()[1] if pq else oq.popleft())()
                emit_AV(kb)

            # deferred PSUM evacuation (run by the NEXT block): O rows cast
            # to bf16 at their target partition offset, both heads' D rows
            # packed into partition 0 (custom ops need base-0 inputs) and
            # inverted in ONE reciprocal; bcast on gpsimd; normalizing
            # multiplies on DVE with partition-aligned operands
            def evac_thunk(p=p, q0=q0, po=po, last=last):
                drow2 = rpool.tile([1, 1024], F32, tag="drow", name="drow2")
                ous = []
                for i in range(2):
                    ou = rpool.tile([128, 512], BF16, tag=f"ou{i}", name="ou")
                    if last:
                        # ScalarE is idle after the final exp: it handles
                        # the O-row evacuation while DVE does D-rows+recip
                        nc.scalar.copy(
                            ou[i * 64:(i + 1) * 64, :], po[i][0:64, :])
                        nc.scalar.copy(
                            drow2[0:1, i * 512:(i + 1) * 512],
                            po[i][64:65, :])
                    else:
                        nc.vector.tensor_copy(
                            ou[i * 64:(i + 1) * 64, :], po[i][0:64, :])
                        nc.vector.tensor_copy(
                            drow2[0:1, i * 512:(i + 1) * 512],
                            po[i][64:65, :])
                    ous.append(ou)
                rcp2 = rpool.tile([1, 1024], F32, tag="rcp", name="rcp2")
                nc.vector.reciprocal_approx_fast(rcp2[:], drow2[:])
                rb = rpool.tile([128, 1024], F32, tag="rb", name="rb")
                nc.gpsimd.partition_broadcast(rb[:], rcp2[0:1, :])
                for i in range(2):
                    nc.vector.tensor_mul(
                        oT[p][i * 64:(i + 1) * 64, q0:q0 + 512],
                        ous[i][i * 64:(i + 1) * 64, :],
                        rb[i * 64:(i + 1) * 64, i * 512:(i + 1) * 512])
            return evac_thunk

        def outproj_co(ch, co, ych):
            ps = fill.tile([128, 512], F32, tag="fill", name="ps_out")
            for p in range(2):
                nc.tensor.matmul(
                    ps[:],
                    wp_sb[:, p * C + co * 128: p * C + (co + 1) * 128],
                    oT[p][:, ch * 512:(ch + 1) * 512],
                    start=(p == 0), stop=(p == 1),
                )
            nc.vector.tensor_copy(ych[:, co * 512:(co + 1) * 512], ps[:])

        def outproj_thunks(ch):
            ych = evac.tile([128, T], BF16, tag="y", name=f"ych{ch}")
            thunks = []
            for co in range(4):
                def mk(co=co):
                    def t():
                        outproj_co(ch, co, ych)
                        if ch == 3 and co == 1:
                            # final chunk: first half leaves early on the
                            # idle sync queue so the last transfer is short
                            nc.sync.dma_start(
                                out[:, ch * 2048: ch * 2048 + 1024],
                                ych[:, 0:1024])
                        elif ch == 3 and co == 3:
                            nc.gpsimd.dma_start(
                                out[:, ch * 2048 + 1024: ch * 2048 + 2048],
                                ych[:, 1024:2048])
                        elif co == 3:
                            nc.gpsimd.dma_start(
                                out[:, ch * 2048:(ch + 1) * 2048], ych[:])
                    return t
                thunks.append(mk())
            return thunks

        # ---- stage 0 pre-work (paced by chunk-0 DMA arrival); only what
        # attnA0 needs — K1/Q1 ride the filler queue ----
        with nc.named_scope("stage0"):
            proj_qk(wk_sb, xkvT, kT[0], 0, 0)
            proj_qk(wq_sb, xqT, qT[0], 0, 0)

        pq = deque()
        oq = deque()

        def proj_halves(key, w_sb, x_sb, dst, p, ch):
            """A projection as TWO filler thunks of 2 matmuls each (same
            key, adjacent in the queue - need-drain runs both in order)
            so one filler pop never exceeds the per-k-tile PE budget."""
            st = {}

            def t1():
                st["ps"] = fill.tile([128, 512], F32, tag="fill",
                                     name=f"ph{p}{ch}")
                for ci in range(2):
                    nc.tensor.matmul(
                        st["ps"][:],
                        wslice(w_sb, ci, p * 128, (p + 1) * 128),
                        xslice(x_sb, ci, ch),
                        start=(ci == 0), stop=False, skip_group_check=True,
                    )

            def t2():
                for ci in range(2, 4):
                    nc.tensor.matmul(
                        st["ps"][:],
                        wslice(w_sb, ci, p * 128, (p + 1) * 128),
                        xslice(x_sb, ci, ch),
                        start=False, stop=(ci == 3), skip_group_check=True,
                    )
                nc.vector.tensor_copy(
                    dst[:, ch * 512:(ch + 1) * 512], st["ps"][:])

            return [(key, t1), (key, t2)]

        pq.extend(proj_halves(("k", 1, 0), wk_sb, xkvT, kT[1], 1, 0))
        pq.extend(proj_halves(("q", 1, 0), wq_sb, xqT, qT[1], 1, 0))

        def stage_thunks(cn):
            return (
                proj_halves(("k", 0, cn), wk_sb, xkvT, kT[0], 0, cn)
                + proj_halves(("k", 1, cn), wk_sb, xkvT, kT[1], 1, cn)
                + proj_halves(("q", 0, cn), wq_sb, xqT, qT[0], 0, cn)
                + proj_halves(("q", 1, cn), wq_sb, xqT, qT[1], 1, cn)
            )

        ev = None
        for c in range(4):
            if c < 3:
                pq.extend(stage_thunks(c + 1))
            with nc.named_scope(f"attnA{c}"):
                ev = attention_block(0, c, pq, oq, pre_evac=ev)
            with nc.named_scope(f"attnB{c}"):
                ev = attention_block(1, c, pq, oq, pre_evac=ev,
                                     last=(c == 3))
            if c < 3:
                oq.extend(outproj_thunks(c))
        # tail: the final chunk's out-proj is split by p so all four p=0
        # halves (only need A3's oT, normalized during B3) sit ready in
        # the PE queue and run concurrently with the B3 evac chain; the
        # p=1 halves + casts follow once the chain's muls land.  PSUM
        # comes from the now-free ps_s pool.
        with nc.named_scope("tail"):
            while oq:
                oq.popleft()()
            ych = evac.tile([128, T], BF16, tag="y", name="ych3f")
            pst = [ps_s.tile([128, 1024], F32, tag="s", name=f"pso{j}")
                   for j in range(2)]

            def ps_slice(co):
                return pst[co // 2][:, (co % 2) * 512:(co % 2 + 1) * 512]

            for co in range(4):
                nc.tensor.matmul(
                    ps_slice(co),
                    wp_sb[:, 0 * C + co * 128: 0 * C + (co + 1) * 128],
                    oT[0][:, 3 * 512:4 * 512],
                    start=True, stop=False, skip_group_check=True,
                )
            ev()
            for co in range(4):
                nc.tensor.matmul(
                    ps_slice(co),
                    wp_sb[:, 1 * C + co * 128: 1 * C + (co + 1) * 128],
                    oT[1][:, 3 * 512:4 * 512],
                    start=False, stop=True, skip_group_check=True,
                )
                nc.vector.tensor_copy(
                    ych[:, co * 512:(co + 1) * 512], ps_slice(co))
                if co == 1:
                    nc.sync.dma_start(
                        out[:, 3 * 2048: 3 * 2048 + 1024], ych[:, 0:1024])
            nc.gpsimd.dma_start(
                out[:, 3 * 2048 + 1024: 4 * 2048], ych[:, 1024:2048])
    return nc


_NC_CACHE = None


def _get_nc():
    global _NC_CACHE
    if _NC_CACHE is None:
        nc = bacc.Bacc("TRN2", target_bir_lowering=False, debug=False,
                       num_devices=8)
        _build(nc)
        nc.compile()
        _NC_CACHE = nc
    return _NC_CACHE


def _shard_inputs(q, kv, Wq, Wkv, Wp):
    v, b, c, h, w = q.shape
    in_maps = []
    for bi in range(b):
        # [v, c, hw] -> [c, T] -> [128p, 4ch, 4ci, 512t] pre-arranged on
        # host so each on-device chunk DMA is contiguous per partition
        def xlay(x):
            xT = x.reshape(v, c, h * w).transpose(1, 0, 2).reshape(c, -1)
            return np.ascontiguousarray(
                xT.reshape(4, 128, 4, 512).transpose(1, 2, 0, 3)
                .reshape(128, -1)).astype(BF16_NP)
        xq = xlay(q[:, bi])
        xkv = xlay(kv[:, bi])
        for g in range(2):
            wq_h = np.ascontiguousarray(
                Wq[:, g * GC:(g + 1) * GC].reshape(4, 128, GC)
                .transpose(1, 0, 2).reshape(128, -1)).astype(BF16_NP)
            wk_h = np.ascontiguousarray(
                Wkv[:, g * GC:(g + 1) * GC].reshape(4, 128, GC)
                .transpose(1, 0, 2).reshape(128, -1)).astype(BF16_NP)
            wv_h = np.ascontiguousarray(
                Wkv[:, c + g * GC:c + (g + 1) * GC].reshape(4, 128, GC)
                .transpose(1, 0, 2).reshape(128, -1)).astype(BF16_NP)
            wp_h = np.ascontiguousarray(
                Wp[g * GC:(g + 1) * GC, :].reshape(2, 128, c)
                .transpose(1, 0, 2).reshape(128, -1)).astype(BF16_NP)
            in_maps.append({
                "xq": xq, "xkv": xkv,
                "wq": wq_h, "wk": wk_h, "wv": wv_h, "wp": wp_h,
            })
    return in_maps


def kernel(q, kv, Wq, bq, Wkv, bkv, Wp, bp, _trace=False):
    q = np.asarray(q, np.float32)
    kv = np.asarray(kv, np.float32)
    v, b, c, h, w = q.shape
    nc = _get_nc()
    in_maps = _shard_inputs(q, kv, np.asarray(Wq, np.float32),
                            np.asarray(Wkv, np.float32),
                            np.asarray(Wp, np.float32))
    res = run_bass_kernel_spmd(nc, in_maps, core_ids=list(range(8)),
                               trace=_trace)
    y = np.empty((v, b, c, h, w), np.float32)
    bp32 = np.asarray(bp, np.float32)
    for bi in range(b):
        y2 = (res.results[bi * 2]["out"].astype(np.float32)
              + res.results[bi * 2 + 1]["out"].astype(np.float32))
        # [128p, 4ch, 4co, 512t] -> [c = co*128+p, T = ch*512+t]
        yT = (y2.reshape(128, 4, 4, 512).transpose(2, 0, 1, 3)
              .reshape(c, v * h * w))
        yT = yT + bp32[:, None]
        y[:, bi] = yT.reshape(c, v, h, w).transpose(1, 0, 2, 3)
    kernel._last_exec_time_ns = res.exec_time_ns
    kernel._last_results = res
    return y


# revision 69
# speedup vs baseline: 1.0474x; 1.0112x over previous
"""Trainium2 Bass kernel: block-causal cross attention (CrossCausalAttention).

Full-input contract: kernel(**inputs) takes the unsharded tensors from
setup_inputs() and returns the full [v, b, c, h, w] output.

Sharding: 8 NeuronCores = 4 batches (data parallel) x 2 head-groups of 4
heads (tensor parallel).  Each core computes a partial y^T [512, 2048] for
its (batch, head-group); the host sums the two head-group partials per batch.

v25 schedule (ScalarE-exp is the pacer; everything else hides under it):
  - Q/K projection fillers are split into TWO half-thunks of 2 matmuls
    each (same key, adjacent in queue) so a single filler pop never
    exceeds the per-k-tile PE budget.
  - Filler thunks are SKIPPED on iterations whose emit_AV lazily projects
    a new V tile: per k-tile the PE budget vs exp is ~1.15us, and
    V-projection (+1us) plus a filler together overload the iteration,
    blow the 2-slot S ring and stall the exp stream.
  - Host pre-arranges x/out to [128p, 4ch, 4ci|4co, 512t] so every chunk
    DMA is one contiguous 4KB-per-partition transfer (128 descriptors;
    DMA-issue time was the startup critical path).
  - No warmup matmuls: stage-0 projections start the moment chunk-0 DMA
    lands; V tiles project lazily inside the attention loop.
  - Attention runs S-matmuls 2 k-tiles ahead of the exp stream, AV one
    behind; e-tile pool is 6 deep so AV jitter never stalls exp.
  - Projections for chunk c+1 and out-proj for chunk c-1 are filler
    thunks popped one per 2 k-tiles inside the attention loop.
  - Each block's PSUM evacuation (O-rows, D-rows, reciprocal, bcast,
    normalize) is DEFERRED into the next block's instruction stream so
    its serial DVE/gpsimd chain overlaps attention instead of stalling
    the chunk boundary; the final block's evac splits the copies onto
    the then-idle ScalarE.
  - Tail: the last chunk's out-proj is p-split - all four p=0 half
    matmuls (needing only A3's already-normalized oT) queue ahead of the
    B3 evac chain and overlap it in free ps_s PSUM; the out DMA leaves
    in two queue-parallel halves.
"""
import sys
from collections import deque

for _p in ("/opt/trn_rl_repo", "/root/.axon_site/_ro/trn_rl_repo"):
    if _p not in sys.path:
        sys.path.append(_p)

import ml_dtypes
import numpy as np

import concourse.bass as bass  # noqa: E402,F401
import concourse.mybir as mybir  # noqa: E402
import concourse.tile as tile  # noqa: E402
from concourse import bacc  # noqa: E402
from concourse.bass_utils import run_bass_kernel_spmd  # noqa: E402

F32 = mybir.dt.float32
BF16 = mybir.dt.bfloat16
BF16_NP = ml_dtypes.bfloat16

V, C, HW = 8, 512, 256
T = V * HW                 # 2048
NHC = 4                    # heads per core
HD = 64
GC = NHC * HD              # 256 channels per head-group
NKT = T // 128             # 16 tk tiles
VW = NHC * (HD + 1)        # 260


def _build(nc):
    from contextlib import ExitStack

    # x layouts are host-prearranged to [128p, 4ch, 4ci, 512t] so each
    # t-chunk DMA is a single contiguous 4KB-per-partition transfer
    # (128 descriptors instead of 512 - DMA issue was the startup
    # critical path).  Same for the output: [128p, 4ch, 4co, 512t].
    xq = nc.dram_tensor("xq", [128, 4 * T], BF16, kind="ExternalInput")
    xkv = nc.dram_tensor("xkv", [128, 4 * T], BF16, kind="ExternalInput")
    wq = nc.dram_tensor("wq", [128, 4 * GC], BF16, kind="ExternalInput")
    wk = nc.dram_tensor("wk", [128, 4 * GC], BF16, kind="ExternalInput")
    wv = nc.dram_tensor("wv", [128, 4 * GC], BF16, kind="ExternalInput")
    wp = nc.dram_tensor("wp", [128, 2 * C], BF16, kind="ExternalInput")
    out = nc.dram_tensor("out", [128, 4 * T], BF16, kind="ExternalOutput")

    with tile.TileContext(nc) as tc, ExitStack() as ctx:
        persist = ctx.enter_context(tc.tile_pool(name="persist", bufs=1))
        epool = ctx.enter_context(tc.tile_pool(name="e", bufs=6))
        rpool = ctx.enter_context(tc.tile_pool(name="r", bufs=4))
        evac = ctx.enter_context(tc.tile_pool(name="evac", bufs=2))
        ps_s = ctx.enter_context(tc.tile_pool(name="ps_s", bufs=2, space="PSUM"))
        po_p = ctx.enter_context(tc.tile_pool(name="po", bufs=2, space="PSUM"))
        fill = ctx.enter_context(tc.tile_pool(name="fill", bufs=2, space="PSUM"))

        # ---- weights: host already laid out [128, 4, GC] / [128, 2, C] ----
        wk_sb = persist.tile([128, 4 * GC], BF16, tag="wk", name="wk_sb")
        nc.scalar.dma_start(wk_sb[:], wk[:])
        wq_sb = persist.tile([128, 4 * GC], BF16, tag="wq", name="wq_sb")
        nc.scalar.dma_start(wq_sb[:], wq[:])
        wv_sb = persist.tile([128, 4 * GC], BF16, tag="wv", name="wv_sb")
        nc.scalar.dma_start(wv_sb[:], wv[:])
        wp_sb = persist.tile([128, 2 * C], BF16, tag="wp", name="wp_sb")
        nc.scalar.dma_start(wp_sb[:], wp[:])

        def wslice(t, ci, lo, hi, cols=GC):
            return t[:, ci * cols + lo: ci * cols + hi]

        # ---- inputs: [128, (ci, T)]; 4 t-chunk DMAs per tensor ----
        xkvT = persist.tile([128, 4 * T], BF16, tag="xkvT", name="xkvT")
        xqT = persist.tile([128, 4 * T], BF16, tag="xqT", name="xqT")
        for c in range(4):
            lo, hi = c * 2048, (c + 1) * 2048
            nc.sync.dma_start(xkvT[:, lo:hi], xkv[:, lo:hi])
            nc.sync.dma_start(xqT[:, lo:hi], xq[:, lo:hi])

        def xslice(x_sb, ci, ch):
            base = ch * 2048 + ci * 512
            return x_sb[:, base: base + 512]

        qT = [persist.tile([128, T], BF16, tag=f"qT{p}", name=f"qT{p}")
              for p in range(2)]
        kT = [persist.tile([128, T], BF16, tag=f"kT{p}", name=f"kT{p}")
              for p in range(2)]
        v_sb = [persist.tile([128, VW], BF16, tag=f"v{tk}", name=f"v{tk}")
                for tk in range(NKT)]
        oT = [persist.tile([128, T], BF16, tag=f"oT{p}", name=f"oT{p}")
              for p in range(2)]

        # ---- per-chunk projection / V-tile emitters ----
        def proj_qk(w_sb, x_sb, dst, p, ch):
            ps = fill.tile([128, 512], F32, tag="fill", name=f"ps_p{p}c{ch}")
            for ci in range(4):
                nc.tensor.matmul(
                    ps[:],
                    wslice(w_sb, ci, p * 128, (p + 1) * 128),
                    xslice(x_sb, ci, ch),
                    start=(ci == 0), stop=(ci == 3),
                )
            nc.vector.tensor_copy(dst[:, ch * 512:(ch + 1) * 512], ps[:])

        def proj_v(tk):
            vt = v_sb[tk]
            nc.vector.memset(
                vt[:].rearrange("p (h x) -> p h x", x=65)[:, :, 64:65]
                .bitcast(mybir.dt.uint16),
                0x3F80)  # bf16 bits of 1.0
            ps = fill.tile([128, 512], F32, tag="fill", name=f"ps_v{tk}")
            for ci in range(4):
                base = (tk // 4) * 2048 + ci * 512 + (tk % 4) * 128
                nc.tensor.matmul(
                    ps[:, 0:GC],
                    xkvT[:, base: base + 128],
                    wslice(wv_sb, ci, 0, GC),
                    start=(ci == 0), stop=(ci == 3),
                )
            nc.vector.tensor_copy(
                vt[:].rearrange("p (h x) -> p h x", x=65)[:, :, 0:64],
                ps[:, 0:GC].rearrange("p (h x) -> p h x", x=64),
            )

        # ---- attention: S runs 2 k-tiles ahead of exp, AV one behind ----
        v_done = [False] * NKT
        v_state = {}

        def v_half1(tk):
            if v_done[tk] or tk in v_state:
                return
            vt = v_sb[tk]
            nc.vector.memset(
                vt[:].rearrange("p (h x) -> p h x", x=65)[:, :, 64:65]
                .bitcast(mybir.dt.uint16), 0x3F80)
            st = {"ps": fill.tile([128, 512], F32, tag="fill",
                                  name=f"psv{tk}")}
            v_state[tk] = st
            for ci in range(2):
                base = (tk // 4) * 2048 + ci * 512 + (tk % 4) * 128
                nc.tensor.matmul(
                    st["ps"][:, 0:GC], xkvT[:, base: base + 128],
                    wslice(wv_sb, ci, 0, GC),
                    start=(ci == 0), stop=False, skip_group_check=True)

        def v_half2(tk):
            if v_done[tk]:
                return
            if tk not in v_state:
                v_half1(tk)
            st = v_state.pop(tk)
            for ci in range(2, 4):
                base = (tk // 4) * 2048 + ci * 512 + (tk % 4) * 128
                nc.tensor.matmul(
                    st["ps"][:, 0:GC], xkvT[:, base: base + 128],
                    wslice(wv_sb, ci, 0, GC),
                    start=False, stop=(ci == 3), skip_group_check=True)
            nc.vector.tensor_copy(
                v_sb[tk][:].rearrange("p (h x) -> p h x", x=65)[:, :, 0:64],
                st["ps"][:, 0:GC].rearrange("p (h x) -> p h x", x=64))
            v_done[tk] = True

        def ensure_v(tk):
            if not v_done[tk]:
                if tk in v_state:
                    v_half2(tk)
                else:
                    proj_v(tk)
                    v_done[tk] = True

        def v_thunks(tks):
            out = []
            for tk in tks:
                out.append((("v", tk), lambda tk=tk: v_half1(tk)))
                out.append((("v", tk), lambda tk=tk: v_half2(tk)))
            return out

        def attention_block(p, qc, pq, oq, pre_evac=None, last=False):
            hA, hB = 2 * p, 2 * p + 1
            q0 = qc * 512
            nfull = 2 * (2 * qc + 1)
            nk = nfull + 2
            po = [po_p.tile([65, 512], F32, tag="po", name=f"po{p}q{qc}{i}")
                  for i in range(2)]
            sq = [None] * nk
            es = [None] * nk

            def emit_S(kb):
                sq[kb] = ps_s.tile([128, 1024], F32, tag="s", name="sps")
                for i, h0 in enumerate((0, 64)):
                    nc.tensor.matmul(
                        sq[kb][:, i * 512:(i + 1) * 512],
                        kT[p][h0:h0 + 64, kb * 128:(kb + 1) * 128],
                        qT[p][h0:h0 + 64, q0:q0 + 512],
                        start=True, stop=True,
                    )

            def emit_ACT(kb):
                if kb < nfull:
                    e = epool.tile([128, 1024], BF16, tag="e", name="e")
                    nc.scalar.activation(
                        e[:], sq[kb][:],
                        mybir.ActivationFunctionType.Exp, scale=0.125)
                else:
                    # boundary key block: only the 2nd half of the queries
                    # may attend.  (K=64 row-base-64 matmuls with N=256
                    # hang TRN2, so S is computed at N=512 and the valid
                    # halves are gathered by the exp's access pattern.)
                    e = epool.tile([128, 512], BF16, tag="e", name="eb")
                    nc.scalar.activation(
                        e[:].rearrange("p (h x) -> p h x", x=256),
                        sq[kb][:].rearrange("p (h x) -> p h x",
                                            x=512)[:, :, 256:512],
                        mybir.ActivationFunctionType.Exp, scale=0.125)
                es[kb] = e
                sq[kb] = None

            def emit_AV(kb):
                ensure_v(kb)
                e = es[kb]
                if kb < nfull:
                    rhs = [e[:, 0:512], e[:, 512:1024]]
                    dst = [po[0][:], po[1][:]]
                else:
                    rhs = [e[:, 0:256], e[:, 256:512]]
                    dst = [po[0][:, 256:512], po[1][:, 256:512]]
                for i, h in enumerate((hA, hB)):
                    nc.tensor.matmul(
                        dst[i],
                        v_sb[kb][:, h * 65: h * 65 + 65],
                        rhs[i],
                        start=(kb == 0), stop=(kb == nk - 1),
                        skip_group_check=True,
                    )
                es[kb] = None

            # selective drain: emit exactly the queued projections this
            # block's S stream depends on; leave the rest for pacing
            need = {("k", p, cc) for cc in range(qc + 1)}
            need.add(("q", p, qc))
            rest = deque()
            while pq:
                key, th = pq.popleft()
                if key in need:
                    th()
                else:
                    rest.append((key, th))
            pq.extend(rest)

            emit_S(0)
            emit_S(1)
            emit_ACT(0)
            # the PREVIOUS block's PSUM evacuation goes here: its serial
            # DVE/gpsimd chain overlaps this block's S/exp stream instead
            # of stalling the chunk boundary (po WAR is released before
            # this block's AV(0) needs the buffers)
            if pre_evac is not None:
                pre_evac()
            for kb in range(nk):
                if kb + 2 < nk:
                    emit_S(kb + 2)
                if kb + 1 < nk:
                    emit_ACT(kb + 1)
                # iterations whose emit_AV lazily projects a new V tile
                # already carry ~1us of extra PE work - adding a filler
                # too overloads the iteration and stalls the exp stream
                if v_done[kb] and kb % 2 == 1 and (pq or oq):
                    (pq.popleft()[1] if pq else oq.popleft())()
                emit_AV(kb)

            # deferred PSUM evacuation (run by the NEXT block): O rows cast
            # to bf16 at their target partition offset, both heads' D rows
            # packed into partition 0 (custom ops need base-0 inputs) and
            # inverted in ONE reciprocal; bcast on gpsimd; normalizing
            # multiplies on DVE with partition-aligned operands
            def evac_thunk(p=p, q0=q0, po=po, last=last):
                drow2 = rpool.tile([1, 1024], F32, tag="drow", name="drow2")
                ous = []
                for i in range(2):
                    ou = rpool.tile([128, 512], BF16, tag=f"ou{i}", name="ou")
                    if last:
                        # ScalarE is idle after the final exp: it handles
                        # the O-row evacuation while DVE does D-rows+recip
                        nc.scalar.copy(
                            ou[i * 64:(i + 1) * 64, :], po[i][0:64, :])
                        nc.scalar.copy(
                            drow2[0:1, i * 512:(i + 1) * 512],
                            po[i][64:65, :])
                    else:
                        nc.vector.tensor_copy(
                            ou[i * 64:(i + 1) * 64, :], po[i][0:64, :])
                        nc.vector.tensor_copy(
                            drow2[0:1, i * 512:(i + 1) * 512],
                            po[i][64:65, :])
                    ous.append(ou)
                rcp2 = rpool.tile([1, 1024], F32, tag="rcp", name="rcp2")
                nc.vector.reciprocal_approx_fast(rcp2[:], drow2[:])
                rb = rpool.tile([128, 1024], F32, tag="rb", name="rb")
                nc.gpsimd.partition_broadcast(rb[:], rcp2[0:1, :])
                for i in range(2):
                    nc.vector.tensor_mul(
                        oT[p][i * 64:(i + 1) * 64, q0:q0 + 512],
                        ous[i][i * 64:(i + 1) * 64, :],
                        rb[i * 64:(i + 1) * 64, i * 512:(i + 1) * 512])
            return evac_thunk

        def outproj_co(ch, co, ych):
            ps = fill.tile([128, 512], F32, tag="fill", name="ps_out")
            for p in range(2):
                nc.tensor.matmul(
                    ps[:],
                    wp_sb[:, p * C + co * 128: p * C + (co + 1) * 128],
                    oT[p][:, ch * 512:(ch + 1) * 512],
                    start=(p == 0), stop=(p == 1),
                )
            nc.vector.tensor_copy(ych[:, co * 512:(co + 1) * 512], ps[:])

        def outproj_thunks(ch):
            ych = evac.tile([128, T], BF16, tag="y", name=f"ych{ch}")
            thunks = []
            for co in range(4):
                def mk(co=co):
                    def t():
                        outproj_co(ch, co, ych)
                        if ch == 3 and co == 1:
                            # final chunk: first half leaves early on the
                            # idle sync queue so the last transfer is short
                            nc.sync.dma_start(
                                out[:, ch * 2048: ch * 2048 + 1024],
                                ych[:, 0:1024])
                        elif ch == 3 and co == 3:
                            nc.gpsimd.dma_start(
                                out[:, ch * 2048 + 1024: ch * 2048 + 2048],
                                ych[:, 1024:2048])
                        elif co == 3:
                            nc.gpsimd.dma_start(
                                out[:, ch * 2048:(ch + 1) * 2048], ych[:])
                    return t
                thunks.append(mk())
            return thunks

        # ---- stage 0 pre-work (paced by chunk-0 DMA arrival); only what
        # attnA0 needs — K1/Q1 ride the filler queue ----
        with nc.named_scope("stage0"):
            proj_qk(wk_sb, xkvT, kT[0], 0, 0)
            proj_qk(wq_sb, xqT, qT[0], 0, 0)

        pq = deque()
        oq = deque()

        def proj_halves(key, w_sb, x_sb, dst, p, ch):
            """A projection as TWO filler thunks of 2 matmuls each (same
            key, adjacent in the queue - need-drain runs both in order)
            so one filler pop never exceeds the per-k-tile PE budget."""
            st = {}

            def t1():
                st["ps"] = fill.tile([128, 512], F32, tag="fill",
                                     name=f"ph{p}{ch}")
                for ci in range(2):
                    nc.tensor.matmul(
                        st["ps"][:],
                        wslice(w_sb, ci, p * 128, (p + 1) * 128),
                        xslice(x_sb, ci, ch),
                        start=(ci == 0), stop=False, skip_group_check=True,
                    )

            def t2():
                for ci in range(2, 4):
                    nc.tensor.matmul(
                        st["ps"][:],
                        wslice(w_sb, ci, p * 128, (p + 1) * 128),
                        xslice(x_sb, ci, ch),
                        start=False, stop=(ci == 3), skip_group_check=True,
                    )
                nc.vector.tensor_copy(
                    dst[:, ch * 512:(ch + 1) * 512], st["ps"][:])

            return [(key, t1), (key, t2)]

        pq.extend(proj_halves(("k", 1, 0), wk_sb, xkvT, kT[1], 1, 0))
        pq.extend(proj_halves(("q", 1, 0), wq_sb, xqT, qT[1], 1, 0))

        def stage_thunks(cn):
            return (
                proj_halves(("k", 0, cn), wk_sb, xkvT, kT[0], 0, cn)
                + proj_halves(("k", 1, cn), wk_sb, xkvT, kT[1], 1, cn)
                + proj_halves(("q", 0, cn), wq_sb, xqT, qT[0], 0, cn)
                + proj_halves(("q", 1, cn), wq_sb, xqT, qT[1], 1, cn)
            )

        ev = None
        for c in range(4):
            if c < 3:
                pq.extend(stage_thunks(c + 1))
                # pre-queue the NEXT chunk's new V tiles as split halves:
                # they spread across this chunk's filler slots instead of
                # piling up lazily at the chunk boundary
                pq.extend(v_thunks(range(4 * c + 4, 4 * c + 8)))
            with nc.named_scope(f"attnA{c}"):
                ev = attention_block(0, c, pq, oq, pre_evac=ev)
            with nc.named_scope(f"attnB{c}"):
                ev = attention_block(1, c, pq, oq, pre_evac=ev,
                                     last=(c == 3))
            if c < 3:
                oq.extend(outproj_thunks(c))
        # tail: the final chunk's out-proj is split by p so all four p=0
        # halves (only need A3's oT, normalized during B3) sit ready in
        # the PE queue and run concurrently with the B3 evac chain; the
        # p=1 halves + casts follow once the chain's muls land.  PSUM
        # comes from the now-free ps_s pool.
        with nc.named_scope("tail"):
            while oq:
                oq.popleft()()
            ych = evac.tile([128, T], BF16, tag="y", name="ych3f")
            pst = [ps_s.tile([128, 1024], F32, tag="s", name=f"pso{j}")
                   for j in range(2)]

            def ps_slice(co):
                return pst[co // 2][:, (co % 2) * 512:(co % 2 + 1) * 512]

            for co in range(4):
                nc.tensor.matmul(
                    ps_slice(co),
                    wp_sb[:, 0 * C + co * 128: 0 * C + (co + 1) * 128],
                    oT[0][:, 3 * 512:4 * 512],
                    start=True, stop=False, skip_group_check=True,
                )
            ev()
            for co in range(4):
                nc.tensor.matmul(
                    ps_slice(co),
                    wp_sb[:, 1 * C + co * 128: 1 * C + (co + 1) * 128],
                    oT[1][:, 3 * 512:4 * 512],
                    start=False, stop=True, skip_group_check=True,
                )
                nc.vector.tensor_copy(
                    ych[:, co * 512:(co + 1) * 512], ps_slice(co))
                if co == 1:
                    nc.sync.dma_start(
                        out[:, 3 * 2048: 3 * 2048 + 1024], ych[:, 0:1024])
            nc.gpsimd.dma_start(
                out[:, 3 * 2048 + 1024: 4 * 2048], ych[:, 1024:2048])
    return nc


_NC_CACHE = None


def _get_nc():
    global _NC_CACHE
    if _NC_CACHE is None:
        nc = bacc.Bacc("TRN2", target_bir_lowering=False, debug=False,
                       num_devices=8)
        _build(nc)
        nc.compile()
        _NC_CACHE = nc
    return _NC_CACHE


def _shard_inputs(q, kv, Wq, Wkv, Wp):
    v, b, c, h, w = q.shape
    in_maps = []
    for bi in range(b):
        # [v, c, hw] -> [c, T] -> [128p, 4ch, 4ci, 512t] pre-arranged on
        # host so each on-device chunk DMA is contiguous per partition
        def xlay(x):
            xT = x.reshape(v, c, h * w).transpose(1, 0, 2).reshape(c, -1)
            return np.ascontiguousarray(
                xT.reshape(4, 128, 4, 512).transpose(1, 2, 0, 3)
                .reshape(128, -1)).astype(BF16_NP)
        xq = xlay(q[:, bi])
        xkv = xlay(kv[:, bi])
        for g in range(2):
            wq_h = np.ascontiguousarray(
                Wq[:, g * GC:(g + 1) * GC].reshape(4, 128, GC)
                .transpose(1, 0, 2).reshape(128, -1)).astype(BF16_NP)
            wk_h = np.ascontiguousarray(
                Wkv[:, g * GC:(g + 1) * GC].reshape(4, 128, GC)
                .transpose(1, 0, 2).reshape(128, -1)).astype(BF16_NP)
            wv_h = np.ascontiguousarray(
                Wkv[:, c + g * GC:c + (g + 1) * GC].reshape(4, 128, GC)
                .transpose(1, 0, 2).reshape(128, -1)).astype(BF16_NP)
            wp_h = np.ascontiguousarray(
                Wp[g * GC:(g + 1) * GC, :].reshape(2, 128, c)
                .transpose(1, 0, 2).reshape(128, -1)).astype(BF16_NP)
            in_maps.append({
                "xq": xq, "xkv": xkv,
                "wq": wq_h, "wk": wk_h, "wv": wv_h, "wp": wp_h,
            })
    return in_maps


def kernel(q, kv, Wq, bq, Wkv, bkv, Wp, bp, _trace=False):
    q = np.asarray(q, np.float32)
    kv = np.asarray(kv, np.float32)
    v, b, c, h, w = q.shape
    nc = _get_nc()
    in_maps = _shard_inputs(q, kv, np.asarray(Wq, np.float32),
                            np.asarray(Wkv, np.float32),
                            np.asarray(Wp, np.float32))
    res = run_bass_kernel_spmd(nc, in_maps, core_ids=list(range(8)),
                               trace=_trace)
    y = np.empty((v, b, c, h, w), np.float32)
    bp32 = np.asarray(bp, np.float32)
    for bi in range(b):
        y2 = (res.results[bi * 2]["out"].astype(np.float32)
              + res.results[bi * 2 + 1]["out"].astype(np.float32))
        # [128p, 4ch, 4co, 512t] -> [c = co*128+p, T = ch*512+t]
        yT = (y2.reshape(128, 4, 4, 512).transpose(2, 0, 1, 3)
              .reshape(c, v * h * w))
        yT = yT + bp32[:, None]
        y[:, bi] = yT.reshape(c, v, h, w).transpose(1, 0, 2, 3)
    kernel._last_exec_time_ns = res.exec_time_ns
    kernel._last_results = res
    return y
